# revision 12
# baseline (speedup 1.0000x reference)
"""nn_DeformUpSample Trainium2 kernel: full inputs in, full output out.

Decomposition (validated numerically in fp32/bf16 against the reference):
  - tile(x,(1,4,1,1)) makes all 4 deform groups sample the same 64 channels,
    so the offset conv folds to Cin=64 and the work splits into 8 independent
    (batch b, group g) units -> one NeuronCore each (core = 4*b + g).
  - per unit on device: 64->27 offset conv (PE, 9 shifted matmuls/tile),
    pointwise offset math (DVE/ACT), per-tap bilinear quad gather via
    dma_gather(transpose=True) from a host-built padded table
    (entry = 2x2 pixel corners x 64ch bf16 = 512B), corner-weight scaling
    (DVE), and a PSUM-accumulated DCN matmul with x-duplicated weights that
    absorbs the 4-corner sum. BatchNorm stats all-reduce across the 8 cores;
    scale+shift+ReLU on device. Host does the final pixel-shuffle assembly.
"""
import sys
import numpy as np
import ml_dtypes

sys.path.insert(0, '/opt/trn_rl_repo')

import concourse.bass as bass  # noqa: E402,F401
import concourse.tile as tile  # noqa: E402
from concourse import bacc, mybir  # noqa: E402
from concourse.bass_utils import run_bass_kernel_spmd  # noqa: E402

F32 = mybir.dt.float32
BF16 = mybir.dt.bfloat16
I16 = mybir.dt.int16
ALU = mybir.AluOpType
AF = mybir.ActivationFunctionType
AX = mybir.AxisListType

S, SS, K, KK, PAD = 2, 4, 3, 9, 1
EPS = 1e-5
H = W = 128
C = 64
N = H * W                # 16384
PQ = 6                   # quad-table zero pad (offsets bounded ~2.6)
Hq = Wq = H + 2 * PQ + 1  # 141
NE = Hq * Wq             # 19881 entries (< int16 max)
XP = H + 2               # conv pad-1 image width
CHUNK = 2048             # psum chunk (positions)
QUAR = 4096              # broadcast granularity
NCH = N // CHUNK         # 8
NW = N // 16             # idx cols per tap
bf16 = ml_dtypes.bfloat16

_CACHE = {}


def _build():
    nc = bacc.Bacc("TRN2", target_bir_lowering=False, debug=False, num_devices=8)

    xp_in = nc.declare_dram_parameter("xp", [C, XP * XP], BF16, isOutput=False)
    qtab = nc.declare_dram_parameter("qtab", [NE, 256], BF16, isOutput=False)
    woff = nc.declare_dram_parameter("woff", [C, KK * 27], BF16, isOutput=False)
    bias = nc.declare_dram_parameter("bias", [27, 1], F32, isOutput=False)
    wdcn = nc.declare_dram_parameter("wdcn", [128, KK * C], BF16, isOutput=False)
    base = nc.declare_dram_parameter("base", [36, QUAR], F32, isOutput=False)
    gb = nc.declare_dram_parameter("gb", [C, 2], F32, isOutput=False)
    outy = nc.declare_dram_parameter("outy", [C, N], F32, isOutput=True)

    ypre_d = nc.dram_tensor("ypre_d", [C, N], F32)
    idxj_d = nc.dram_tensor("idxj_d", [KK, N], I16)
    cc_in = nc.dram_tensor("cc_in", [C, 2], F32)
    cc_out = nc.dram_tensor("cc_out", [C, 2], F32, addr_space="Shared")

    with tile.TileContext(nc) as tc:
        with tc.tile_pool(name="persist", bufs=1) as pp:
            wdcn_sb = pp.tile([128, KK * C], BF16)
            nc.sync.dma_start(wdcn_sb[:, :], wdcn[:, :])
            gb_sb = pp.tile([C, 2], F32)
            nc.sync.dma_start(gb_sb[:, :], gb[:, :])
            abro = pp.tile([36, N], BF16)        # corner-weight rows, j-order
            idxw = pp.tile([128, KK * NW], I16)  # wrapped gather indices
            sums = pp.tile([C, NCH], F32)
            ssqs = pp.tile([C, NCH], F32)

            with tc.tile_pool(name="mid", bufs=1) as mp:
                off_dy = mp.tile([36, QUAR], BF16)
                off_dx = mp.tile([36, QUAR], BF16)
                off_mk = mp.tile([36, QUAR], BF16)

                # ---- phase 1: offset conv ------------------------------
                with (
                    tc.tile_pool(name="convp", bufs=1) as cp,
                    tc.tile_pool(name="pscv", bufs=2, space="PSUM") as pscv,
                ):
                    xp_sb = cp.tile([C, XP * XP], BF16)
                    nc.sync.dma_start(xp_sb[:, :], xp_in[:, :])
                    woff_sb = cp.tile([C, KK * 27], BF16)
                    nc.sync.dma_start(woff_sb[:, :], woff[:, :])
                    bias_sb = cp.tile([27, 1], F32)
                    nc.sync.dma_start(bias_sb[:, :], bias[:, :])

                    off27 = cp.tile([27, N], BF16)
                    xp3 = xp_sb[:, :].rearrange("p (h w) -> p h w", h=XP)
                    for nt in range(N // 512):
                        r0 = 4 * nt
                        ps = pscv.tile([27, 512], F32)
                        for t in range(KK):
                            kh, kw = t // K, t % K
                            rhs = xp3[:, r0 + kh:r0 + kh + 4, kw:kw + W]
                            nc.tensor.matmul(
                                ps[:, :], woff_sb[:, 27 * t:27 * (t + 1)],
                                rhs, start=(t == 0), stop=(t == KK - 1))
                        nc.vector.tensor_scalar(
                            off27[:, 512 * nt:512 * (nt + 1)], ps[:, :],
                            bias_sb[:, 0:1], None, ALU.add)

                    # scatter -> interleaved [36, 4096] (p = 4r+s, c = j//4)
                    o3 = off27[:, :].rearrange("p (c s) -> p c s", s=4)
                    for qty, dst in ((0, off_dy), (1, off_dx), (2, off_mk)):
                        d3 = dst[:, :].rearrange("(r s) c -> r s c", s=4)
                        for s in range(4):
                            nc.sync.dma_start(d3[:, s, :],
                                              o3[9 * qty:9 * (qty + 1), :, s])

                # ---- phase 2: pointwise --------------------------------
                with (
                    tc.tile_pool(name="pwp", bufs=1) as wp,
                    tc.tile_pool(name="scr16", bufs=1) as s16,
                    tc.tile_pool(name="scr8", bufs=1) as s8,
                ):
                    HQ = QUAR // 2
                    I32 = mybir.dt.int32

                    def t16(tag):
                        return s16.tile([36, HQ], F32, tag=tag, name=tag)

                    def t8(tag, dt=BF16):
                        return s8.tile([36, HQ], dt, tag=tag, name=tag)

                    def floor_frac(src, ef_tag, fr_tag):
                        ri = t8("ri", I32)
                        nc.vector.tensor_copy(ri[:, :], src)
                        rf = t8("rf")
                        nc.vector.tensor_copy(rf[:, :], ri[:, :])
                        gt = t8("gt")
                        nc.vector.tensor_tensor(gt[:, :], rf[:, :], src,
                                                ALU.is_gt)
                        e = t8(ef_tag)
                        nc.vector.tensor_tensor(e[:, :], rf[:, :], gt[:, :],
                                                ALU.subtract)
                        fr = t16(fr_tag)
                        nc.vector.tensor_tensor(fr[:, :], src, e[:, :],
                                                ALU.subtract)
                        return e, fr

                    for h2 in range(2):
                        ch = slice(HQ * h2, HQ * (h2 + 1))
                        base_sb = wp.tile([36, HQ], F32, tag="base",
                                          name="base")
                        nc.sync.dma_start(base_sb[:, :], base[:, ch])

                        ef, fr_y = floor_frac(off_dy[:, ch], "ef", "fry")
                        ff, fr_x = floor_frac(off_dx[:, ch], "ff", "frx")

                        m = t8("m")
                        nc.scalar.activation(m[:, :], off_mk[:, ch],
                                             AF.Sigmoid)
                        cly = t8("cly")
                        nc.scalar.activation(cly[:, :], fr_y[:, :], AF.Copy,
                                             bias=1.0, scale=-1.0)
                        clx = t8("clx")
                        nc.scalar.activation(clx[:, :], fr_x[:, :], AF.Copy,
                                             bias=1.0, scale=-1.0)
                        u = t8("u")
                        nc.vector.tensor_tensor(u[:, :], m[:, :], cly[:, :],
                                                ALU.mult)
                        v = t8("v")
                        nc.vector.tensor_tensor(v[:, :], m[:, :], fr_y[:, :],
                                                ALU.mult)
                        for cc, (uu, ll) in enumerate(
                                ((u, clx), (u, fr_x), (v, clx), (v, fr_x))):
                            a = s8.tile([36, HQ], BF16, tag="acorn",
                                        name="acorn", bufs=2)
                            nc.vector.tensor_tensor(a[:, :], uu[:, :],
                                                    ll[:, :], ALU.mult)
                            a3 = a[:, :].rearrange("(r s) c -> r s c", s=4)
                            d3 = abro[9 * cc:9 * (cc + 1),
                                      8192 * h2:8192 * (h2 + 1)].rearrange(
                                "r (c s) -> r c s", s=4)
                            for s in range(4):
                                nc.sync.dma_start(d3[:, :, s], a3[:, s, :])

                        idx_t = t16("fry")
                        nc.vector.scalar_tensor_tensor(
                            idx_t[:, :], ef[:, :], float(Wq), base_sb[:, :],
                            ALU.mult, ALU.add)
                        idx_f = t16("frx")
                        nc.vector.tensor_tensor(idx_f[:, :], idx_t[:, :],
                                                ff[:, :], ALU.add)
                        idx16 = s8.tile([36, HQ], I16, tag="i16", name="i16")
                        nc.vector.tensor_copy(idx16[:, :], idx_f[:, :])

                        i3 = idx16[:, :].rearrange("(r s) c -> r s c", s=4)
                        j3 = idxj_d[:, 8192 * h2:8192 * (h2 + 1)].rearrange(
                            "r (c s) -> r c s", s=4)
                        for s in range(4):
                            nc.sync.dma_start(j3[:, :, s], i3[:, s, :])

                    # 16-wrap + 8x replicate
                    for t in range(KK):
                        src = idxj_d[t:t + 1, :].rearrange(
                            "r (c l) -> r l c", l=16)
                        nc.sync.dma_start(
                            idxw[0:16, NW * t:NW * (t + 1)], src)
                    for mrep in range(1, 8):
                        nc.sync.dma_start(
                            idxw[16 * mrep:16 * (mrep + 1), :], idxw[0:16, :])

            # ---- phase 3: gather + scale + DCN matmul ------------------
            with (
                tc.tile_pool(name="qpool", bufs=6) as qpool,
                tc.tile_pool(name="wpool", bufs=2) as wpool,
                tc.tile_pool(name="ybp", bufs=2) as ybp,
                tc.tile_pool(name="psdcn", bufs=2, space="PSUM") as psdcn,
            ):
                for Q in range(4):
                    pss = [psdcn.tile([C, CHUNK], F32, tag="ps", name="ps")
                           for _ in range(2)]
                    for t in range(KK):
                        wt = wpool.tile([128, 2, QUAR], BF16, tag="wt")
                        for cc in range(4):
                            src = abro[9 * cc + t:9 * cc + t + 1,
                                       QUAR * Q:QUAR * (Q + 1)]
                            nc.sync.dma_start(
                                wt[64 * (cc % 2):64 * (cc % 2) + 64,
                                   cc // 2, :],
                                src.unsqueeze(1).broadcast_to([1, 64, QUAR]))
                        for hf in range(2):
                            j0 = QUAR * Q + CHUNK * hf
                            q = qpool.tile([128, 2, CHUNK], BF16, tag="q")
                            nc.gpsimd.dma_gather(
                                out_ap=q[:, :, :],
                                in_ap=qtab[:, :],
                                idxs_ap=idxw[:, NW * t + j0 // 16:
                                             NW * t + (j0 + CHUNK) // 16],
                                num_idxs=CHUNK, num_idxs_reg=CHUNK,
                                elem_size=256, transpose=True,
                                single_packet=False)
                            nc.vector.tensor_tensor(
                                q[:, :, :], q[:, :, :],
                                wt[:, :, CHUNK * hf:CHUNK * (hf + 1)],
                                ALU.mult)
                            for pl in range(2):
                                for j in range(CHUNK // 512):
                                    nc.tensor.matmul(
                                        pss[hf][:, 512 * j:512 * (j + 1)],
                                        wdcn_sb[:, C * t:C * (t + 1)],
                                        q[:, pl, 512 * j:512 * (j + 1)],
                                        start=(t == 0 and pl == 0),
                                        stop=(t == KK - 1 and pl == 1))
                    for hf in range(2):
                        ch = 2 * Q + hf
                        yb = ybp.tile([C, CHUNK], F32, tag="yb")
                        nc.scalar.activation(yb[:, :], pss[hf][:, :], AF.Copy,
                                             accum_out=sums[:, ch:ch + 1])
                        sq = ybp.tile([C, CHUNK], BF16, tag="sq")
                        nc.vector.scalar_tensor_tensor(
                            sq[:, :], yb[:, :], 1.0, yb[:, :],
                            ALU.mult, ALU.mult,
                            accum_out=ssqs[:, ch:ch + 1])
                        nc.sync.dma_start(
                            ypre_d[:, CHUNK * ch:CHUNK * (ch + 1)], yb[:, :])

                # ---- BN stats + collective ----------------------------
                st = pp.tile([C, 2], F32)
                nc.vector.tensor_reduce(st[:, 0:1], sums[:, :], op=ALU.add,
                                        axis=AX.X)
                nc.vector.tensor_reduce(st[:, 1:2], ssqs[:, :], op=ALU.add,
                                        axis=AX.X)
                nc.sync.dma_start(cc_in[:, :], st[:, :])
                nc.gpsimd.collective_compute(
                    "AllReduce", ALU.add, replica_groups=[list(range(8))],
                    ins=[cc_in[:, :]], outs=[cc_out[:, :]])
                rt = pp.tile([C, 2], F32)
                nc.sync.dma_start(rt[:, :], cc_out[:, :])

                CNT = 1.0 / (8 * N)
                mean = pp.tile([C, 1], F32)
                nc.vector.tensor_scalar(mean[:, :], rt[:, 0:1], CNT, None,
                                        ALU.mult)
                ex2 = pp.tile([C, 1], F32)
                nc.vector.tensor_scalar(ex2[:, :], rt[:, 1:2], CNT, None,
                                        ALU.mult)
                var = pp.tile([C, 1], F32)
                nc.vector.scalar_tensor_tensor(var[:, :], mean[:, :], -1.0,
                                               mean[:, :], ALU.mult, ALU.mult)
                nc.vector.scalar_tensor_tensor(var[:, :], ex2[:, :], EPS,
                                               var[:, :], ALU.add, ALU.add)
                sd = pp.tile([C, 1], F32)
                nc.scalar.activation(sd[:, :], var[:, :], AF.Sqrt)
                inv = pp.tile([C, 1], F32)
                nc.vector.reciprocal(inv[:, :], sd[:, :])
                scale = pp.tile([C, 1], F32)
                nc.vector.tensor_tensor(scale[:, :], gb_sb[:, 0:1],
                                        inv[:, :], ALU.mult)
                shift = pp.tile([C, 1], F32)
                nc.vector.scalar_tensor_tensor(shift[:, :], mean[:, :], -1.0,
                                               scale[:, :], ALU.mult,
                                               ALU.mult)
                nc.vector.tensor_tensor(shift[:, :], gb_sb[:, 1:2],
                                        shift[:, :], ALU.add)

                # ---- normalize + relu + out ---------------------------
                for ch in range(NCH):
                    t_in = ybp.tile([C, CHUNK], F32, tag="fin")
                    nc.sync.dma_start(
                        t_in[:, :], ypre_d[:, CHUNK * ch:CHUNK * (ch + 1)])
                    t_out = ybp.tile([C, CHUNK], F32, tag="fout")
                    nc.scalar.activation(t_out[:, :], t_in[:, :], AF.Relu,
                                         scale=scale[:, 0:1],
                                         bias=shift[:, 0:1])
                    nc.sync.dma_start(
                        outy[:, CHUNK * ch:CHUNK * (ch + 1)], t_out[:, :])

    nc.compile()
    return nc


# --------------------------------------------------------------------------
# host prep
# --------------------------------------------------------------------------
def _prep(x, w_offset, b_offset, w_dcn, gamma, beta):
    w_fold = w_offset.reshape(108, SS, C, K, K).sum(axis=1)

    hh = np.arange(N, dtype=np.int64) // W
    ww = np.arange(N, dtype=np.int64) % W
    khs = np.arange(KK) // K - PAD
    kws = np.arange(KK) % K - PAD
    base = np.empty((36, QUAR), np.float32)
    jj = np.arange(N)
    for k in range(KK):
        bk = (hh + khs[k] + PQ) * Wq + (ww + kws[k] + PQ)
        base[4 * k + jj % 4, jj // 4] = bk
    gbv = np.stack([gamma, beta], axis=1).astype(np.float32)

    per_batch = []
    for b in range(2):
        xb = x[b]
        xpad = np.zeros((C, Hq + 1, Wq + 1), np.float32)
        xpad[:, PQ:PQ + H, PQ:PQ + W] = xb
        t = np.empty((NE, 4, C), bf16)
        t[:, 0] = xpad[:, 0:Hq, 0:Wq].reshape(C, NE).T
        t[:, 1] = xpad[:, 0:Hq, 1:Wq + 1].reshape(C, NE).T
        t[:, 2] = xpad[:, 1:Hq + 1, 0:Wq].reshape(C, NE).T
        t[:, 3] = xpad[:, 1:Hq + 1, 1:Wq + 1].reshape(C, NE).T
        qt = np.ascontiguousarray(t.reshape(NE, 256))
        xp = np.zeros((C, XP, XP), bf16)
        xp[:, 1:-1, 1:-1] = xb.astype(bf16)
        per_batch.append((qt, np.ascontiguousarray(xp.reshape(C, XP * XP))))

    in_maps = []
    for c in range(8):
        b, g = c // 4, c % 4
        sel = ([18 * g + 2 * k for k in range(KK)] +
               [18 * g + 2 * k + 1 for k in range(KK)] +
               [72 + 9 * g + k for k in range(KK)])
        wsel = w_fold[sel]                                    # [27, 64, 3, 3]
        wof = np.empty((C, KK * 27), bf16)
        for t in range(KK):
            wof[:, 27 * t:27 * (t + 1)] = wsel[:, :, t // K, t % K].T
        wg = w_dcn.reshape(SS, C, C, KK)[g]                   # [o, c, p]
        wd = np.empty((128, KK * C), bf16)
        for t in range(KK):
            lh = wg[:, :, t].T                                # [c, o]
            wd[0:C, C * t:C * (t + 1)] = lh
            wd[C:128, C * t:C * (t + 1)] = lh
        qt, xp = per_batch[b]
        in_maps.append({
            "xp": xp, "qtab": qt, "woff": wof,
            "bias": np.ascontiguousarray(
                b_offset[sel].astype(np.float32)[:, None]),
            "wdcn": wd, "base": base, "gb": gbv,
        })
    return in_maps


def kernel(x, w_offset, b_offset, w_dcn, gamma, beta):
    x = np.asarray(x, np.float32)
    in_maps = _prep(x, np.asarray(w_offset, np.float32),
                    np.asarray(b_offset, np.float32),
                    np.asarray(w_dcn, np.float32),
                    np.asarray(gamma, np.float32),
                    np.asarray(beta, np.float32))
    if "nc" not in _CACHE:
        _CACHE["nc"] = _build()
    nc = _CACHE["nc"]
    res = run_bass_kernel_spmd(nc, in_maps, core_ids=list(range(8)))
    _CACHE["last"] = (nc, in_maps)

    ys = np.zeros((2, SS, C, H, W), np.float32)
    for c in range(8):
        ys[c // 4, c % 4] = res.results[c]["outy"].reshape(C, H, W)
    y = ys.reshape(2, S, S, C, H, W).transpose(0, 3, 4, 1, 5, 2)
    return np.ascontiguousarray(y.reshape(2, C, H * S, W * S))


# revision 16
# speedup vs baseline: 1.0605x; 1.0605x over previous
"""nn_DeformUpSample Trainium2 kernel: full inputs in, full output out.

Decomposition (validated numerically in fp32/bf16 against the reference):
  - tile(x,(1,4,1,1)) makes all 4 deform groups sample the same 64 channels,
    so the offset conv folds to Cin=64 and the work splits into 8 independent
    (batch b, group g) units -> one NeuronCore each (core = 4*b + g).
  - per unit on device: 64->27 offset conv (PE, 9 shifted matmuls/tile),
    pointwise offset math (DVE/ACT), per-tap bilinear quad gather via
    dma_gather(transpose=True) from a host-built padded table
    (entry = 2x2 pixel corners x 64ch bf16 = 512B), corner-weight scaling
    (DVE), and a PSUM-accumulated DCN matmul with x-duplicated weights that
    absorbs the 4-corner sum. BatchNorm stats all-reduce across the 8 cores;
    scale+shift+ReLU on device. Host does the final pixel-shuffle assembly.
"""
import sys
import numpy as np
import ml_dtypes

sys.path.insert(0, '/opt/trn_rl_repo')

import concourse.bass as bass  # noqa: E402,F401
import concourse.tile as tile  # noqa: E402
from concourse import bacc, mybir  # noqa: E402
from concourse.bass_utils import run_bass_kernel_spmd  # noqa: E402

F32 = mybir.dt.float32
BF16 = mybir.dt.bfloat16
I16 = mybir.dt.int16
ALU = mybir.AluOpType
AF = mybir.ActivationFunctionType
AX = mybir.AxisListType

S, SS, K, KK, PAD = 2, 4, 3, 9, 1
EPS = 1e-5
H = W = 128
C = 64
N = H * W                # 16384
PQ = 6                   # quad-table zero pad (offsets bounded ~2.6)
Hq = Wq = H + 2 * PQ + 1  # 141
NE = Hq * Wq             # 19881 entries (< int16 max)
XP = H + 2               # conv pad-1 image width
CHUNK = 2048             # psum chunk (positions)
QUAR = 4096              # broadcast granularity
NCH = N // CHUNK         # 8
NW = N // 16             # idx cols per tap
bf16 = ml_dtypes.bfloat16

_CACHE = {}


def _build():
    nc = bacc.Bacc("TRN2", target_bir_lowering=False, debug=False, num_devices=8)

    xp_in = nc.declare_dram_parameter("xp", [C, XP * XP], BF16, isOutput=False)
    xt_in = nc.declare_dram_parameter("xt", [(Hq + 1) * (Wq + 1), C], BF16,
                                      isOutput=False)
    woff = nc.declare_dram_parameter("woff", [C, KK * 27], BF16, isOutput=False)
    bias = nc.declare_dram_parameter("bias", [27, 1], F32, isOutput=False)
    wdcn = nc.declare_dram_parameter("wdcn", [128, KK * C], BF16, isOutput=False)
    base = nc.declare_dram_parameter("base", [36, QUAR], F32, isOutput=False)
    gb = nc.declare_dram_parameter("gb", [C, 2], F32, isOutput=False)
    outy = nc.declare_dram_parameter("outy", [C, N], F32, isOutput=True)

    qtab = nc.dram_tensor("qtab_d", [NE, 256], BF16)
    ypre_d = nc.dram_tensor("ypre_d", [C, N], F32)
    idxj_d = nc.dram_tensor("idxj_d", [KK, N], I16)
    cc_in = nc.dram_tensor("cc_in", [C, 2], F32)
    cc_out = nc.dram_tensor("cc_out", [C, 2], F32, addr_space="Shared")

    with tile.TileContext(nc) as tc:
        with tc.tile_pool(name="persist", bufs=1) as pp:
            # build the quad table: entry(y,x) = [x_t(y,x), x_t(y,x+1),
            #   x_t(y+1,x), x_t(y+1,x+1)] -- 4 contiguous DRAM->DRAM copies
            xt3 = xt_in[:, :].rearrange("(y x) c -> y x c", x=Wq + 1)
            qt3 = qtab[:, :].rearrange("(y x) (j c) -> y x j c", x=Wq, j=4)
            for jj, (dyj, dxj) in enumerate(
                    ((0, 0), (0, 1), (1, 0), (1, 1))):
                nc.sync.dma_start(
                    qt3[:, :, jj, :],
                    xt3[dyj:dyj + Hq, dxj:dxj + Wq, :])

            wdcn_sb = pp.tile([128, KK * C], BF16)
            nc.sync.dma_start(wdcn_sb[:, :], wdcn[:, :])
            gb_sb = pp.tile([C, 2], F32)
            nc.sync.dma_start(gb_sb[:, :], gb[:, :])
            abro = pp.tile([36, N], BF16)        # corner-weight rows, j-order
            idxw = pp.tile([128, KK * NW], I16)  # wrapped gather indices
            sums = pp.tile([C, NCH], F32)
            ssqs = pp.tile([C, NCH], F32)

            with tc.tile_pool(name="mid", bufs=1) as mp:
                off_dy = mp.tile([36, QUAR], BF16)
                off_dx = mp.tile([36, QUAR], BF16)
                off_mk = mp.tile([36, QUAR], BF16)

                # ---- phase 1: offset conv ------------------------------
                with (
                    tc.tile_pool(name="convp", bufs=1) as cp,
                    tc.tile_pool(name="pscv", bufs=2, space="PSUM") as pscv,
                ):
                    xp_sb = cp.tile([C, XP * XP], BF16)
                    nc.sync.dma_start(xp_sb[:, :], xp_in[:, :])
                    woff_sb = cp.tile([C, KK * 27], BF16)
                    nc.sync.dma_start(woff_sb[:, :], woff[:, :])
                    bias_sb = cp.tile([27, 1], F32)
                    nc.sync.dma_start(bias_sb[:, :], bias[:, :])

                    off27 = cp.tile([27, N], BF16)
                    xp3 = xp_sb[:, :].rearrange("p (h w) -> p h w", h=XP)
                    for nt in range(N // 512):
                        r0 = 4 * nt
                        ps = pscv.tile([27, 512], F32)
                        for t in range(KK):
                            kh, kw = t // K, t % K
                            rhs = xp3[:, r0 + kh:r0 + kh + 4, kw:kw + W]
                            nc.tensor.matmul(
                                ps[:, :], woff_sb[:, 27 * t:27 * (t + 1)],
                                rhs, start=(t == 0), stop=(t == KK - 1))
                        nc.vector.tensor_scalar(
                            off27[:, 512 * nt:512 * (nt + 1)], ps[:, :],
                            bias_sb[:, 0:1], None, ALU.add)

                    # scatter -> interleaved [36, 4096] (p = 4r+s, c = j//4)
                    o3 = off27[:, :].rearrange("p (c s) -> p c s", s=4)
                    for qty, dst in ((0, off_dy), (1, off_dx), (2, off_mk)):
                        d3 = dst[:, :].rearrange("(r s) c -> r s c", s=4)
                        for s in range(4):
                            nc.sync.dma_start(d3[:, s, :],
                                              o3[9 * qty:9 * (qty + 1), :, s])

                # ---- phase 2: pointwise --------------------------------
                with (
                    tc.tile_pool(name="pwp", bufs=1) as wp,
                    tc.tile_pool(name="scr16", bufs=1) as s16,
                    tc.tile_pool(name="scr8", bufs=1) as s8,
                ):
                    HQ = QUAR // 2
                    I32 = mybir.dt.int32

                    def t16(tag):
                        return s16.tile([36, HQ], F32, tag=tag, name=tag)

                    def t8(tag, dt=BF16):
                        return s8.tile([36, HQ], dt, tag=tag, name=tag)

                    def floor_frac(src, ef_tag, fr_tag):
                        ri = t8("ri", I32)
                        nc.vector.tensor_copy(ri[:, :], src)
                        rf = t8("rf")
                        nc.vector.tensor_copy(rf[:, :], ri[:, :])
                        gt = t8("gt")
                        nc.vector.tensor_tensor(gt[:, :], rf[:, :], src,
                                                ALU.is_gt)
                        e = t8(ef_tag)
                        nc.vector.tensor_tensor(e[:, :], rf[:, :], gt[:, :],
                                                ALU.subtract)
                        fr = t16(fr_tag)
                        nc.vector.tensor_tensor(fr[:, :], src, e[:, :],
                                                ALU.subtract)
                        return e, fr

                    for h2 in range(2):
                        ch = slice(HQ * h2, HQ * (h2 + 1))
                        base_sb = wp.tile([36, HQ], F32, tag="base",
                                          name="base")
                        nc.sync.dma_start(base_sb[:, :], base[:, ch])

                        ef, fr_y = floor_frac(off_dy[:, ch], "ef", "fry")
                        ff, fr_x = floor_frac(off_dx[:, ch], "ff", "frx")

                        m = t8("m")
                        nc.scalar.activation(m[:, :], off_mk[:, ch],
                                             AF.Sigmoid)
                        cly = t8("cly")
                        nc.scalar.activation(cly[:, :], fr_y[:, :], AF.Copy,
                                             bias=1.0, scale=-1.0)
                        clx = t8("clx")
                        nc.scalar.activation(clx[:, :], fr_x[:, :], AF.Copy,
                                             bias=1.0, scale=-1.0)
                        u = t8("u")
                        nc.vector.tensor_tensor(u[:, :], m[:, :], cly[:, :],
                                                ALU.mult)
                        v = t8("v")
                        nc.vector.tensor_tensor(v[:, :], m[:, :], fr_y[:, :],
                                                ALU.mult)
                        for cc, (uu, ll) in enumerate(
                                ((u, clx), (u, fr_x), (v, clx), (v, fr_x))):
                            a = s8.tile([36, HQ], BF16, tag="acorn",
                                        name="acorn", bufs=2)
                            nc.vector.tensor_tensor(a[:, :], uu[:, :],
                                                    ll[:, :], ALU.mult)
                            a3 = a[:, :].rearrange("(r s) c -> r s c", s=4)
                            d3 = abro[9 * cc:9 * (cc + 1),
                                      8192 * h2:8192 * (h2 + 1)].rearrange(
                                "r (c s) -> r c s", s=4)
                            for s in range(4):
                                nc.sync.dma_start(d3[:, :, s], a3[:, s, :])

                        idx_t = t16("fry")
                        nc.vector.scalar_tensor_tensor(
                            idx_t[:, :], ef[:, :], float(Wq), base_sb[:, :],
                            ALU.mult, ALU.add)
                        idx_f = t16("frx")
                        nc.vector.tensor_tensor(idx_f[:, :], idx_t[:, :],
                                                ff[:, :], ALU.add)
                        idx16 = s8.tile([36, HQ], I16, tag="i16", name="i16")
                        nc.vector.tensor_copy(idx16[:, :], idx_f[:, :])

                        i3 = idx16[:, :].rearrange("(r s) c -> r s c", s=4)
                        j3 = idxj_d[:, 8192 * h2:8192 * (h2 + 1)].rearrange(
                            "r (c s) -> r c s", s=4)
                        for s in range(4):
                            nc.sync.dma_start(j3[:, :, s], i3[:, s, :])

                    # 16-wrap + 8x replicate
                    for t in range(KK):
                        src = idxj_d[t:t + 1, :].rearrange(
                            "r (c l) -> r l c", l=16)
                        nc.sync.dma_start(
                            idxw[0:16, NW * t:NW * (t + 1)], src)
                    for mrep in range(1, 8):
                        nc.sync.dma_start(
                            idxw[16 * mrep:16 * (mrep + 1), :], idxw[0:16, :])

            # ---- phase 3: gather + scale + DCN matmul ------------------
            with (
                tc.tile_pool(name="qpool", bufs=6) as qpool,
                tc.tile_pool(name="wpool", bufs=2) as wpool,
                tc.tile_pool(name="ybp", bufs=2) as ybp,
                tc.tile_pool(name="psdcn", bufs=2, space="PSUM") as psdcn,
            ):
                for Q in range(4):
                    pss = [psdcn.tile([C, CHUNK], F32, tag="ps", name="ps")
                           for _ in range(2)]
                    for t in range(KK):
                        wt = wpool.tile([128, 2, QUAR], BF16, tag="wt")
                        for cc in range(4):
                            src = abro[9 * cc + t:9 * cc + t + 1,
                                       QUAR * Q:QUAR * (Q + 1)]
                            nc.sync.dma_start(
                                wt[64 * (cc % 2):64 * (cc % 2) + 64,
                                   cc // 2, :],
                                src.unsqueeze(1).broadcast_to([1, 64, QUAR]))
                        for hf in range(2):
                            j0 = QUAR * Q + CHUNK * hf
                            q = qpool.tile([128, 2, CHUNK], BF16, tag="q")
                            nc.gpsimd.dma_gather(
                                out_ap=q[:, :, :],
                                in_ap=qtab[:, :],
                                idxs_ap=idxw[:, NW * t + j0 // 16:
                                             NW * t + (j0 + CHUNK) // 16],
                                num_idxs=CHUNK, num_idxs_reg=CHUNK,
                                elem_size=256, transpose=True,
                                single_packet=False)
                            nc.vector.tensor_tensor(
                                q[:, :, :], q[:, :, :],
                                wt[:, :, CHUNK * hf:CHUNK * (hf + 1)],
                                ALU.mult)
                            for pl in range(2):
                                for j in range(CHUNK // 512):
                                    nc.tensor.matmul(
                                        pss[hf][:, 512 * j:512 * (j + 1)],
                                        wdcn_sb[:, C * t:C * (t + 1)],
                                        q[:, pl, 512 * j:512 * (j + 1)],
                                        start=(t == 0 and pl == 0),
                                        stop=(t == KK - 1 and pl == 1))
                    for hf in range(2):
                        ch = 2 * Q + hf
                        yb = ybp.tile([C, CHUNK], F32, tag="yb")
                        nc.scalar.activation(yb[:, :], pss[hf][:, :], AF.Copy,
                                             accum_out=sums[:, ch:ch + 1])
                        sq = ybp.tile([C, CHUNK], BF16, tag="sq")
                        nc.vector.scalar_tensor_tensor(
                            sq[:, :], yb[:, :], 1.0, yb[:, :],
                            ALU.mult, ALU.mult,
                            accum_out=ssqs[:, ch:ch + 1])
                        nc.sync.dma_start(
                            ypre_d[:, CHUNK * ch:CHUNK * (ch + 1)], yb[:, :])

                # ---- BN stats + collective ----------------------------
                st = pp.tile([C, 2], F32)
                nc.vector.tensor_reduce(st[:, 0:1], sums[:, :], op=ALU.add,
                                        axis=AX.X)
                nc.vector.tensor_reduce(st[:, 1:2], ssqs[:, :], op=ALU.add,
                                        axis=AX.X)
                nc.sync.dma_start(cc_in[:, :], st[:, :])
                nc.gpsimd.collective_compute(
                    "AllReduce", ALU.add, replica_groups=[list(range(8))],
                    ins=[cc_in[:, :]], outs=[cc_out[:, :]])
                rt = pp.tile([C, 2], F32)
                nc.sync.dma_start(rt[:, :], cc_out[:, :])

                CNT = 1.0 / (8 * N)
                mean = pp.tile([C, 1], F32)
                nc.vector.tensor_scalar(mean[:, :], rt[:, 0:1], CNT, None,
                                        ALU.mult)
                ex2 = pp.tile([C, 1], F32)
                nc.vector.tensor_scalar(ex2[:, :], rt[:, 1:2], CNT, None,
                                        ALU.mult)
                var = pp.tile([C, 1], F32)
                nc.vector.scalar_tensor_tensor(var[:, :], mean[:, :], -1.0,
                                               mean[:, :], ALU.mult, ALU.mult)
                nc.vector.scalar_tensor_tensor(var[:, :], ex2[:, :], EPS,
                                               var[:, :], ALU.add, ALU.add)
                sd = pp.tile([C, 1], F32)
                nc.scalar.activation(sd[:, :], var[:, :], AF.Sqrt)
                inv = pp.tile([C, 1], F32)
                nc.vector.reciprocal(inv[:, :], sd[:, :])
                scale = pp.tile([C, 1], F32)
                nc.vector.tensor_tensor(scale[:, :], gb_sb[:, 0:1],
                                        inv[:, :], ALU.mult)
                shift = pp.tile([C, 1], F32)
                nc.vector.scalar_tensor_tensor(shift[:, :], mean[:, :], -1.0,
                                               scale[:, :], ALU.mult,
                                               ALU.mult)
                nc.vector.tensor_tensor(shift[:, :], gb_sb[:, 1:2],
                                        shift[:, :], ALU.add)

                # ---- normalize + relu + out ---------------------------
                for ch in range(NCH):
                    t_in = ybp.tile([C, CHUNK], F32, tag="fin")
                    nc.sync.dma_start(
                        t_in[:, :], ypre_d[:, CHUNK * ch:CHUNK * (ch + 1)])
                    t_out = ybp.tile([C, CHUNK], F32, tag="fout")
                    nc.scalar.activation(t_out[:, :], t_in[:, :], AF.Relu,
                                         scale=scale[:, 0:1],
                                         bias=shift[:, 0:1])
                    nc.sync.dma_start(
                        outy[:, CHUNK * ch:CHUNK * (ch + 1)], t_out[:, :])

    nc.compile()
    return nc


# --------------------------------------------------------------------------
# host prep
# --------------------------------------------------------------------------
def _prep(x, w_offset, b_offset, w_dcn, gamma, beta):
    w_fold = w_offset.reshape(108, SS, C, K, K).sum(axis=1)

    hh = np.arange(N, dtype=np.int64) // W
    ww = np.arange(N, dtype=np.int64) % W
    khs = np.arange(KK) // K - PAD
    kws = np.arange(KK) % K - PAD
    base = np.empty((36, QUAR), np.float32)
    jj = np.arange(N)
    for k in range(KK):
        bk = (hh + khs[k] + PQ) * Wq + (ww + kws[k] + PQ)
        base[4 * k + jj % 4, jj // 4] = bk
    gbv = np.stack([gamma, beta], axis=1).astype(np.float32)

    per_batch = []
    for b in range(2):
        xb16 = x[b].astype(bf16)
        xt = np.zeros((Hq + 1, Wq + 1, C), bf16)
        xt[PQ:PQ + H, PQ:PQ + W] = xb16.transpose(1, 2, 0)
        xp = np.zeros((C, XP, XP), bf16)
        xp[:, 1:-1, 1:-1] = xb16
        per_batch.append((
            np.ascontiguousarray(xt.reshape((Hq + 1) * (Wq + 1), C)),
            np.ascontiguousarray(xp.reshape(C, XP * XP))))

    in_maps = []
    for c in range(8):
        b, g = c // 4, c % 4
        sel = ([18 * g + 2 * k for k in range(KK)] +
               [18 * g + 2 * k + 1 for k in range(KK)] +
               [72 + 9 * g + k for k in range(KK)])
        wsel = w_fold[sel]                                    # [27, 64, 3, 3]
        wof = np.empty((C, KK * 27), bf16)
        for t in range(KK):
            wof[:, 27 * t:27 * (t + 1)] = wsel[:, :, t // K, t % K].T
        wg = w_dcn.reshape(SS, C, C, KK)[g]                   # [o, c, p]
        wd = np.empty((128, KK * C), bf16)
        for t in range(KK):
            lh = wg[:, :, t].T                                # [c, o]
            wd[0:C, C * t:C * (t + 1)] = lh
            wd[C:128, C * t:C * (t + 1)] = lh
        xt, xp = per_batch[b]
        in_maps.append({
            "xp": xp, "xt": xt, "woff": wof,
            "bias": np.ascontiguousarray(
                b_offset[sel].astype(np.float32)[:, None]),
            "wdcn": wd, "base": base, "gb": gbv,
        })
    return in_maps


def kernel(x, w_offset, b_offset, w_dcn, gamma, beta):
    x = np.asarray(x, np.float32)
    in_maps = _prep(x, np.asarray(w_offset, np.float32),
                    np.asarray(b_offset, np.float32),
                    np.asarray(w_dcn, np.float32),
                    np.asarray(gamma, np.float32),
                    np.asarray(beta, np.float32))
    if "nc" not in _CACHE:
        _CACHE["nc"] = _build()
    nc = _CACHE["nc"]
    res = run_bass_kernel_spmd(nc, in_maps, core_ids=list(range(8)))
    _CACHE["last"] = (nc, in_maps)

    ys = np.zeros((2, SS, C, H, W), np.float32)
    for c in range(8):
        ys[c // 4, c % 4] = res.results[c]["outy"].reshape(C, H, W)
    y = ys.reshape(2, S, S, C, H, W).transpose(0, 3, 4, 1, 5, 2)
    return np.ascontiguousarray(y.reshape(2, C, H * S, W * S))


# revision 24
# speedup vs baseline: 7.3135x; 6.8960x over previous
"""nn_DeformUpSample Trainium2 kernel: full inputs in, full output out.

Decomposition (validated numerically in fp32/bf16 against the reference):
  - tile(x,(1,4,1,1)) makes all 4 deform groups sample the same 64 channels,
    so the offset conv folds to Cin=64 and the work splits into 8 independent
    (batch b, group g) units -> one NeuronCore each (core = 4*b + g).
  - per unit on device: 64->27 offset conv (PE) emitted directly in an
    s-interleaved 108-row psum layout (4 stride-4 position groups packed in
    the M dim), pointwise offset math (DVE/ACT), per-tap bilinear quad
    gathers via dma_gather(transpose=True) from a device-built padded table
    (entry = 2x2 pixel corners x 64ch bf16 = 512B, corner-blocked), corner
    weight scaling (DVE), and a PSUM-accumulated DCN matmul whose contraction
    absorbs the 4-corner bilinear sum. Positions are streamed in
    (s = j%4)-major order so weight rows broadcast contiguously; the output
    DMA undoes the permutation. BatchNorm stats all-reduce across the 8
    cores; scale+shift+ReLU on device. Host does the pixel-shuffle assembly.
"""
import sys
import numpy as np
import ml_dtypes

sys.path.insert(0, '/opt/trn_rl_repo')

import concourse.bass as bass  # noqa: E402,F401
import concourse.tile as tile  # noqa: E402
from concourse import bacc, mybir  # noqa: E402
from concourse.bass_utils import run_bass_kernel_spmd  # noqa: E402

F32 = mybir.dt.float32
BF16 = mybir.dt.bfloat16
I16 = mybir.dt.int16
I32 = mybir.dt.int32
ALU = mybir.AluOpType
AF = mybir.ActivationFunctionType
AX = mybir.AxisListType

S, SS, K, KK, PAD = 2, 4, 3, 9, 1
EPS = 1e-5
H = W = 128
C = 64
N = H * W                 # 16384
PQ = 6                    # quad-table zero pad (offsets bounded ~2.6)
Hq = Wq = H + 2 * PQ + 1  # 141
NE = Hq * Wq              # 19881 entries (< int16 max)
XP = H + 2                # conv pad-1 image width
CHUNK = 2048              # psum chunk (stream positions)
QUAR = 4096               # per-s stream block
NCH = N // CHUNK          # 8
NW = N // 16              # idxw cols per tap
HQ = QUAR // 2            # pointwise half width
# corner order within an entry: x-major: (y0x0, y1x0, y0x1, y1x1)
CORNERS = ((0, 0), (1, 0), (0, 1), (1, 1))
bf16 = ml_dtypes.bfloat16

_CACHE = {}


def _build(skip=()):
    nc = bacc.Bacc("TRN2", target_bir_lowering=False, debug=False, num_devices=8)

    xp_in = nc.declare_dram_parameter("xp", [C, XP * XP], BF16, isOutput=False)
    xt_in = nc.declare_dram_parameter("xt", [(Hq + 1) * (Wq + 1), C], BF16,
                                      isOutput=False)
    woff = nc.declare_dram_parameter("woff", [C, 36 * 108], BF16,
                                     isOutput=False)
    bias = nc.declare_dram_parameter("bias", [108, 1], F32, isOutput=False)
    wdcn = nc.declare_dram_parameter("wdcn", [128, 2 * KK * C], BF16,
                                     isOutput=False)
    base = nc.declare_dram_parameter("base", [36, QUAR], F32, isOutput=False)
    gb = nc.declare_dram_parameter("gb", [C, 2], F32, isOutput=False)
    outy = nc.declare_dram_parameter("outy", [C, N], F32, isOutput=True)

    qtab = nc.dram_tensor("qtab_d", [NE, 256], BF16)
    idx16d = nc.dram_tensor("idx16d", [36, QUAR], I16)
    cc_in = nc.dram_tensor("cc_in", [C, 2], F32)
    cc_out = nc.dram_tensor("cc_out", [C, 2], F32, addr_space="Shared")

    with tile.TileContext(nc) as tc:
        with tc.tile_pool(name="persist", bufs=1) as pp:
            # quad table: entry elem = 128*(c//32) + 32*corner + c%32
            xt4 = xt_in[:, :].rearrange("(y x) (f c) -> y x f c",
                                        x=Wq + 1, f=2)
            qt5 = qtab[:, :].rearrange("(y x) (f j c) -> y x f j c",
                                       x=Wq, f=2, j=4)
            for jj, (dyj, dxj) in enumerate(CORNERS):
                nc.sync.dma_start(
                    qt5[:, :, :, jj, :],
                    xt4[dyj:dyj + Hq, dxj:dxj + Wq, :, :])

            wdcn_sb = pp.tile([128, 2 * KK * C], BF16)
            nc.sync.dma_start(wdcn_sb[:, :], wdcn[:, :])
            gb_sb = pp.tile([C, 2], F32)
            nc.sync.dma_start(gb_sb[:, :], gb[:, :])
            # corner-weight tiles: rows yc*36 + 4k + s
            aP0 = pp.tile([72, QUAR], BF16)   # x0 corners (a00 | a10)
            aP1 = pp.tile([72, QUAR], BF16)   # x1 corners (a01 | a11)
            idx16f = pp.tile([36, QUAR], I16)
            idxw = pp.tile([128, KK * NW], I16)
            sums = pp.tile([C, NCH], F32)
            ssqs = pp.tile([C, NCH], F32)
            ypre_sb = pp.tile([C, N], BF16)

            with tc.tile_pool(name="mid", bufs=1) as mp:
                offI = mp.tile([108, QUAR], BF16)

                # ---- phase 1: offset conv (s-interleaved M) ------------
                with (
                    tc.tile_pool(name="convp", bufs=1) as cp,
                    tc.tile_pool(name="pscv", bufs=2, space="PSUM") as pscv,
                ):
                    xp_sb = cp.tile([C, XP * XP], BF16)
                    nc.sync.dma_start(xp_sb[:, :], xp_in[:, :])
                    woff_sb = cp.tile([C, 36 * 108], BF16)
                    nc.sync.dma_start(woff_sb[:, :], woff[:, :])
                    bias_sb = cp.tile([108, 1], F32)
                    nc.sync.dma_start(bias_sb[:, :], bias[:, :])

                    xp3 = xp_sb[:, :].rearrange("p (h w) -> p h w", h=XP)
                    for cb in range(8 if "conv" not in skip else 0):
                        ps = pscv.tile([108, 512], F32)
                        h0 = 16 * cb
                        for s4 in range(4):
                            for t in range(KK):
                                kh, kw = t // K, t % K
                                rhs = xp3[:, h0 + kh:h0 + kh + 16,
                                          kw + s4:kw + s4 + 125:4]
                                lcol = 108 * (9 * s4 + t)
                                nc.tensor.matmul(
                                    ps[:, :], woff_sb[:, lcol:lcol + 108],
                                    rhs, start=(s4 == 0 and t == 0),
                                    stop=(s4 == 3 and t == KK - 1))
                        nc.vector.tensor_scalar(
                            offI[:, 512 * cb:512 * (cb + 1)], ps[:, :],
                            bias_sb[:, 0:1], None, ALU.add)
                    if "conv" in skip:
                        nc.gpsimd.memset(offI[:, :], 0.0)

                # ---- phase 2: pointwise --------------------------------
                with (
                    tc.tile_pool(name="pwp", bufs=1) as wp,
                    tc.tile_pool(name="s72", bufs=1) as s72,
                    tc.tile_pool(name="s36", bufs=1) as s36,
                ):
                    def t72(tag, dt=BF16):
                        return s72.tile([72, HQ], dt, tag=tag, name=tag)

                    def t36(tag, dt=BF16):
                        return s36.tile([36, HQ], dt, tag=tag, name=tag)

                    for h2 in range(2):
                        ch = slice(HQ * h2, HQ * (h2 + 1))
                        base_sb = wp.tile([36, HQ], F32, tag="base",
                                          name="base")
                        nc.sync.dma_start(base_sb[:, :], base[:, ch])
                        mk0 = t36("mk0")
                        nc.sync.dma_start(mk0[:, :], offI[72:108, ch])

                        # floor + frac for dy (rows 0:36) & dx (rows 36:72)
                        ri = t72("ri", I32)
                        nc.vector.tensor_copy(ri[:, :], offI[0:72, ch])
                        rf = t72("rf")
                        nc.vector.tensor_copy(rf[:, :], ri[:, :])
                        gt = t72("gt")
                        nc.vector.tensor_tensor(gt[:, :], rf[:, :],
                                                offI[0:72, ch], ALU.is_gt)
                        e = t72("e")
                        nc.vector.tensor_tensor(e[:, :], rf[:, :], gt[:, :],
                                                ALU.subtract)
                        fr = t72("fr")
                        nc.vector.tensor_tensor(fr[:, :], offI[0:72, ch],
                                                e[:, :], ALU.subtract)
                        cl = t72("cl")
                        nc.scalar.activation(cl[:, :], fr[:, :], AF.Copy,
                                             bias=1.0, scale=-1.0)
                        m = t36("m")
                        nc.scalar.activation(m[:, :], mk0[:, :], AF.Sigmoid)

                        uv = t72("uv")
                        nc.vector.tensor_tensor(uv[0:36, :], m[:, :],
                                                cl[0:36, :], ALU.mult)
                        vt = t36("vt")
                        nc.vector.tensor_tensor(vt[:, :], m[:, :],
                                                fr[0:36, :], ALU.mult)
                        nc.sync.dma_start(uv[36:72, :], vt[:, :])
                        cx2 = t72("cx2")
                        nc.sync.dma_start(cx2[0:36, :], cl[36:72, :])
                        nc.sync.dma_start(cx2[36:72, :], cl[36:72, :])
                        lx2 = t72("lx2")
                        nc.sync.dma_start(lx2[0:36, :], fr[36:72, :])
                        nc.sync.dma_start(lx2[36:72, :], fr[36:72, :])
                        nc.vector.tensor_tensor(aP0[:, ch], uv[:, :],
                                                cx2[:, :], ALU.mult)
                        nc.vector.tensor_tensor(aP1[:, ch], uv[:, :],
                                                lx2[:, :], ALU.mult)

                        # entry indices
                        ff0 = t36("ff0")
                        nc.sync.dma_start(ff0[:, :], e[36:72, :])
                        idx_t = t36("idxt", F32)
                        nc.vector.scalar_tensor_tensor(
                            idx_t[:, :], e[0:36, :], float(Wq),
                            base_sb[:, :], ALU.mult, ALU.add)
                        idx_f = t36("idxf", F32)
                        nc.vector.tensor_tensor(idx_f[:, :], idx_t[:, :],
                                                ff0[:, :], ALU.add)
                        nc.vector.tensor_copy(idx16f[:, ch], idx_f[:, :])

                    # 16-wrap + 8x replicate (stream order i = 4096*s + c)
                    nc.sync.dma_start(idx16d[:, :], idx16f[:, :])
                    for t in range(KK):
                        for s4 in range(4):
                            src = idx16d[4 * t + s4:4 * t + s4 + 1,
                                         :].rearrange(
                                "r (c l) -> r l c", l=16)
                            nc.sync.dma_start(
                                idxw[0:16, NW * t + 256 * s4:
                                     NW * t + 256 * (s4 + 1)], src)
                    for mrep in range(1, 8):
                        nc.sync.dma_start(
                            idxw[16 * mrep:16 * (mrep + 1), :], idxw[0:16, :])

            # ---- phase 3: gather + scale + DCN matmul ------------------
            with (
                tc.tile_pool(name="qpool", bufs=3) as qpool,
                tc.tile_pool(name="wpool", bufs=2) as wpool,
                tc.tile_pool(name="ybp", bufs=2) as ybp,
                tc.tile_pool(name="psdcn", bufs=2, space="PSUM") as psdcn,
            ):
                for s4 in range(4):
                    pss = [psdcn.tile([C, CHUNK], F32, tag="ps", name="ps")
                           for _ in range(2)]
                    for t in range(KK):
                        wt = wpool.tile([128, QUAR], BF16, tag="wt")
                        if "bro" in skip:
                            nc.gpsimd.memset(wt[:, :], 0.0)
                        else:
                            for xc, ap in ((0, aP0), (1, aP1)):
                                src = ap[4 * t + s4:4 * t + s4 + 37:36, :]
                                nc.gpsimd.dma_start(
                                    wt[64 * xc:64 * (xc + 1), :],
                                    src.unsqueeze(1).broadcast_to(
                                        [2, 32, QUAR]))
                        q = qpool.tile([128, 2, QUAR], BF16, tag="q")
                        if "gather" not in skip:
                            nc.gpsimd.dma_gather(
                                out_ap=q[:, :, :],
                                in_ap=qtab[:, :],
                                idxs_ap=idxw[:, NW * t + 256 * s4:
                                             NW * t + 256 * (s4 + 1)],
                                num_idxs=QUAR, num_idxs_reg=QUAR,
                                elem_size=256, transpose=True,
                                single_packet=False)
                        for pl in range(2):
                            if "mul" not in skip:
                                nc.vector.tensor_tensor(
                                    q[:, pl, :], q[:, pl, :], wt[:, :],
                                    ALU.mult)
                            if "mm" in skip:
                                continue
                            for hf in range(2):
                                for j in range(CHUNK // 512):
                                    c0 = CHUNK * hf + 512 * j
                                    nc.tensor.matmul(
                                        pss[hf][:, 512 * j:512 * (j + 1)],
                                        wdcn_sb[:, C * (2 * t + pl):
                                                C * (2 * t + pl + 1)],
                                        q[:, pl, c0:c0 + 512],
                                        start=(t == 0 and pl == 0),
                                        stop=(t == KK - 1 and pl == 1))
                    for hf in range(2):
                        ch = 2 * s4 + hf
                        ysl = ypre_sb[:, CHUNK * ch:CHUNK * (ch + 1)]
                        nc.scalar.activation(ysl, pss[hf][:, :], AF.Copy,
                                             accum_out=sums[:, ch:ch + 1])
                        sq = ybp.tile([C, CHUNK], BF16, tag="sq")
                        nc.vector.scalar_tensor_tensor(
                            sq[:, :], ysl, 1.0, ysl, ALU.mult, ALU.mult,
                            accum_out=ssqs[:, ch:ch + 1])

                # ---- BN stats + collective ----------------------------
                st = pp.tile([C, 2], F32)
                nc.vector.tensor_reduce(st[:, 0:1], sums[:, :], op=ALU.add,
                                        axis=AX.X)
                nc.vector.tensor_reduce(st[:, 1:2], ssqs[:, :], op=ALU.add,
                                        axis=AX.X)
                nc.sync.dma_start(cc_in[:, :], st[:, :])
                if "cc" not in skip:
                    nc.gpsimd.collective_compute(
                        "AllReduce", ALU.add, replica_groups=[list(range(8))],
                        ins=[cc_in[:, :]], outs=[cc_out[:, :]])
                else:
                    nc.sync.dma_start(cc_out[:, :], cc_in[:, :])
                rt = pp.tile([C, 2], F32)
                nc.sync.dma_start(rt[:, :], cc_out[:, :])

                CNT = 1.0 / (8 * N)
                mean = pp.tile([C, 1], F32)
                nc.vector.tensor_scalar(mean[:, :], rt[:, 0:1], CNT, None,
                                        ALU.mult)
                ex2 = pp.tile([C, 1], F32)
                nc.vector.tensor_scalar(ex2[:, :], rt[:, 1:2], CNT, None,
                                        ALU.mult)
                var = pp.tile([C, 1], F32)
                nc.vector.scalar_tensor_tensor(var[:, :], mean[:, :], -1.0,
                                               mean[:, :], ALU.mult, ALU.mult)
                nc.vector.scalar_tensor_tensor(var[:, :], ex2[:, :], EPS,
                                               var[:, :], ALU.add, ALU.add)
                sd = pp.tile([C, 1], F32)
                nc.scalar.activation(sd[:, :], var[:, :], AF.Sqrt)
                inv = pp.tile([C, 1], F32)
                nc.vector.reciprocal(inv[:, :], sd[:, :])
                scale = pp.tile([C, 1], F32)
                nc.vector.tensor_tensor(scale[:, :], gb_sb[:, 0:1],
                                        inv[:, :], ALU.mult)
                shift = pp.tile([C, 1], F32)
                nc.vector.scalar_tensor_tensor(shift[:, :], mean[:, :], -1.0,
                                               scale[:, :], ALU.mult,
                                               ALU.mult)
                nc.vector.tensor_tensor(shift[:, :], gb_sb[:, 1:2],
                                        shift[:, :], ALU.add)

                # ---- normalize + relu + out (un-permute stream order) --
                for ch in range(NCH):
                    t_out = ybp.tile([C, CHUNK], F32, tag="fout")
                    nc.scalar.activation(
                        t_out[:, :],
                        ypre_sb[:, CHUNK * ch:CHUNK * (ch + 1)], AF.Relu,
                        scale=scale[:, 0:1], bias=shift[:, 0:1])
                    nc.sync.dma_start(
                        outy[:, CHUNK * ch:CHUNK * (ch + 1)], t_out[:, :])

    nc.compile()
    return nc


# --------------------------------------------------------------------------
# host prep
# --------------------------------------------------------------------------
def _prep(x, w_offset, b_offset, w_dcn, gamma, beta):
    w_fold = w_offset.reshape(108, SS, C, K, K).sum(axis=1)

    # base grid in interleaved layout: row 4k+s, col c  (j = 4c+s)
    hh = np.arange(N, dtype=np.int64) // W
    ww = np.arange(N, dtype=np.int64) % W
    khs = np.arange(KK) // K - PAD
    kws = np.arange(KK) % K - PAD
    base = np.empty((36, QUAR), np.float32)
    jj = np.arange(N)
    for k in range(KK):
        bk = (hh + khs[k] + PQ) * Wq + (ww + kws[k] + PQ)
        base[4 * k + jj % 4, jj // 4] = bk
    gbv = np.stack([gamma, beta], axis=1).astype(np.float32)

    per_batch = []
    for b in range(2):
        xb16 = x[b].astype(bf16)
        xt = np.zeros((Hq + 1, Wq + 1, C), bf16)
        xt[PQ:PQ + H, PQ:PQ + W] = xb16.transpose(1, 2, 0)
        xp = np.zeros((C, XP, XP), bf16)
        xp[:, 1:-1, 1:-1] = xb16
        per_batch.append((
            np.ascontiguousarray(xt.reshape((Hq + 1) * (Wq + 1), C)),
            np.ascontiguousarray(xp.reshape(C, XP * XP))))

    rk = np.arange(KK)
    in_maps = []
    for c in range(8):
        b, g = c // 4, c % 4
        sel = ([18 * g + 2 * k for k in range(KK)] +
               [18 * g + 2 * k + 1 for k in range(KK)] +
               [72 + 9 * g + k for k in range(KK)])
        wsel = w_fold[sel]                                    # [27, 64, 3, 3]
        # conv lhsT variants [64, 108] per (s, tap): qty rows at 36q+4r+s
        wof = np.zeros((C, 36 * 108), bf16)
        for s in range(4):
            for t in range(KK):
                lh = np.zeros((C, 108), np.float32)
                for qty in range(3):
                    lh[:, 36 * qty + 4 * rk + s] = \
                        wsel[9 * qty:9 * (qty + 1), :, t // K, t % K].T
                wof[:, 108 * (9 * s + t):108 * (9 * s + t + 1)] = lh
        bias108 = np.zeros((108, 1), np.float32)
        bsel = b_offset[sel]
        for qty in range(3):
            for r in range(KK):
                bias108[36 * qty + 4 * r:36 * qty + 4 * r + 4, 0] = \
                    bsel[9 * qty + r]
        wg = w_dcn.reshape(SS, C, C, KK)[g]                   # [o, c, p]
        wd = np.empty((128, 2 * KK * C), bf16)
        for t in range(KK):
            for pl in range(2):
                lh = wg[:, 32 * pl:32 * pl + 32, t].T         # [32c, o]
                wd[:, C * (2 * t + pl):C * (2 * t + pl + 1)] = np.tile(
                    lh, (4, 1))
        xt, xp = per_batch[b]
        in_maps.append({
            "xp": xp, "xt": xt, "woff": wof, "bias": bias108,
            "wdcn": wd, "base": base, "gb": gbv,
        })
    return in_maps


def kernel(x, w_offset, b_offset, w_dcn, gamma, beta):
    x = np.asarray(x, np.float32)
    in_maps = _prep(x, np.asarray(w_offset, np.float32),
                    np.asarray(b_offset, np.float32),
                    np.asarray(w_dcn, np.float32),
                    np.asarray(gamma, np.float32),
                    np.asarray(beta, np.float32))
    if "nc" not in _CACHE:
        _CACHE["nc"] = _build()
    nc = _CACHE["nc"]
    res = run_bass_kernel_spmd(nc, in_maps, core_ids=list(range(8)))
    _CACHE["last"] = (nc, in_maps)

    ys = np.zeros((2, SS, C, H, W), np.float32)
    for c in range(8):
        yp = res.results[c]["outy"].reshape(C, 4, QUAR)   # stream i = 4096s+c
        ys[c // 4, c % 4] = np.ascontiguousarray(
            yp.transpose(0, 2, 1)).reshape(C, H, W)       # j = 4c+s
    y = ys.reshape(2, S, S, C, H, W).transpose(0, 3, 4, 1, 5, 2)
    return np.ascontiguousarray(y.reshape(2, C, H * S, W * S))
